# revision 1
# baseline (speedup 1.0000x reference)
"""ExLlama transformer layer (GPTQ int4) on 8 TRN2 NeuronCores, tensor-parallel.

Self-contained: hardcodes shapes from the problem spec.
  B=1, S=2048, HID=4096, INTER=11008, HEADS=32, HD=128, GS=128.

Sharding (SPMD, identical program per core, per-core data slices):
  - q/k/v column-sharded (4 heads per core), attention head-parallel
  - o row-sharded -> partial sums -> AllReduce -> h2 (replicated)
  - gate/up column-sharded over padded INTER (8*1408=11264, zero-padded)
  - down row-sharded -> partials (+h2/8) -> ReduceScatter -> per-core out slice

In-kernel layout is transposed: activations live as x^T [feature, seq] so
GPTQ weight tiles dequantize straight into matmul lhsT [k, o] layout.
Zero-points are folded as a rank-G matmul correction with host-precomputed
-(z+1)*sc.  Matmuls run fp16 (attention path) / bf16 (MLP path), PSUM fp32.
"""
import sys

sys.path.insert(0, "/opt/trn_rl_repo")

import numpy as np

S = 2048
HID = 4096
HD = 128
GS = 128
INTER = 11008
NCORES = 8
IPC = 1408                      # padded inter features per core
IPAD = IPC * NCORES             # 11264
NKT = HID // 128                # 32 k-tiles over HID
NIT = IPC // 128                # 11 k-tiles over per-core inter
OPC = HID // NCORES             # 512 out features per core (qkv), 4 heads
NHC = OPC // HD                 # 4 heads per core
CHUNK = 512
NCHUNK = S // CHUNK             # 4
NST = S // 128                  # 16 s-tiles
SCALE = 1.0 / float(np.sqrt(HD))
EPS = 1e-6
NEG = -30000.0

_BUILD_CACHE = {}


def _build(debug=False):
    import concourse.bacc as bacc
    import concourse.mybir as mybir
    import concourse.tile as tile
    import ml_dtypes

    dt = mybir.dt
    F32, F16, BF16, I32 = dt.float32, dt.float16, dt.bfloat16, dt.int32
    Alu = mybir.AluOpType
    Act = mybir.ActivationFunctionType

    nc = bacc.Bacc("TRN2", target_bir_lowering=False, num_devices=NCORES)

    # ---------------- external I/O ----------------
    hT_d = nc.dram_tensor("hT", [HID, S], F32, kind="ExternalInput")
    cosT_d = nc.dram_tensor("cosT", [HD, S], F16, kind="ExternalInput")
    sinfT_d = nc.dram_tensor("sinfT", [HD, S], F16, kind="ExternalInput")
    ln1_d = nc.dram_tensor("ln1", [HID], F32, kind="ExternalInput")
    ln2_d = nc.dram_tensor("ln2", [HID], F32, kind="ExternalInput")
    qw_qkv_d = nc.dram_tensor("qw_qkv", [12 * 16, NKT * 128], I32, kind="ExternalInput")
    sc_qkv_d = nc.dram_tensor("sc_qkv", [12 * NKT, 128], F16, kind="ExternalInput")
    nz_qkv_d = nc.dram_tensor("nz_qkv", [32, 3 * OPC], F16, kind="ExternalInput")
    qw_o_d = nc.dram_tensor("qw_o", [NKT * 16, NHC * 128], I32, kind="ExternalInput")
    sc_o_d = nc.dram_tensor("sc_o", [NKT * NHC, 128], F16, kind="ExternalInput")
    nz_o_d = nc.dram_tensor("nz_o", [OPC // GS, HID], F16, kind="ExternalInput")
    qw_gu_d = nc.dram_tensor("qw_gu", [2 * NIT * 16, NKT * 128], I32, kind="ExternalInput")
    sc_gu_d = nc.dram_tensor("sc_gu", [2 * NIT * NKT, 128], BF16, kind="ExternalInput")
    nz_gu_d = nc.dram_tensor("nz_gu", [32, 2 * IPC], BF16, kind="ExternalInput")
    qw_dn_d = nc.dram_tensor("qw_dn", [NKT * 16, NIT * 128], I32, kind="ExternalInput")
    sc_dn_d = nc.dram_tensor("sc_dn", [NKT * NIT, 128], BF16, kind="ExternalInput")
    nz_dn_d = nc.dram_tensor("nz_dn", [NIT, HID], BF16, kind="ExternalInput")

    outT_d = nc.dram_tensor("outT", [OPC, S], F32, kind="ExternalOutput")
    dbg = {}
    if debug:
        for nm, shp in [("x1T_dbg", [HID, S]), ("qT_dbg", [HD, S]),
                        ("kT_dbg", [HD, S]), ("oT_dbg", [OPC, S]),
                        ("p1_dbg", [HID, S]), ("x2T_dbg", [HID, S]),
                        ("yT_dbg", [IPC, S])]:
            dbg[nm] = nc.dram_tensor(nm, shp, F32, kind="ExternalOutput")

    # ---------------- inline constants ----------------
    p = np.arange(128)
    sh_c = nc.inline_tensor(((p % 8) * 4).astype(np.int32)[:, None], name="shc")
    id16_c = nc.inline_tensor(np.eye(128, dtype=np.float16), name="id16")
    ones16_c = nc.inline_tensor(np.ones((128, 1), np.float16), name="ones16")
    onesbf_c = nc.inline_tensor(np.ones((128, 1), ml_dtypes.bfloat16),
                                name="onesbf")
    # wide selector: eslw[:, j] = 1 iff j == 31; slice [31-t : 63-t] gives a
    # [128, 32] tile whose column t is all-ones (rowsum-per-group lhsT).
    eslw = np.zeros((128, 63), np.float16)
    eslw[:, 31] = 1.0
    esw16_c = nc.inline_tensor(eslw, name="esw16")
    eswbf_c = nc.inline_tensor(eslw.astype(ml_dtypes.bfloat16), name="eswbf")
    # wide causal mask, S^T layout: mkw[k, j] with qq = j-384: 0 if qq >= k
    # else NEG.  Slice [384-128*dd : 896-128*dd] = mask for diag offset dd.
    j = np.arange(896)
    mkw = np.where((j[None, :] - 384) >= p[:, None], 0.0, NEG)
    mask_c = nc.inline_tensor(mkw.astype(ml_dtypes.bfloat16), name="maskc")

    # Host repacks qw into [notile*16, nkt*128] with rows o*16+r, cols kt*128+c
    # (k-tiles contiguous per out-tile) and sc into [notile*nkt, 128] with rows
    # o*nkt+g.  One 3-dim DMA then loads a multi-k-tile sub-slab.
    def rep_src(qw_ap, ot, kt0, nsub, nkt):
        sl = qw_ap[ot * 16:(ot + 1) * 16,
                   kt0 * 128:(kt0 + nsub) * 128]
        return sl.unsqueeze(1).broadcast_to([16, 8, nsub * 128])

    def sc_src(sc_ap, ot, kt0, nsub, nkt):
        sl = sc_ap[ot * nkt + kt0: ot * nkt + kt0 + nsub, :]
        return sl.rearrange("g c -> (g c)").unsqueeze(0).unsqueeze(0) \
                 .broadcast_to([1, 128, nsub * 128])

    with tile.TileContext(nc) as tc:
        ctx_pools = []

        def open_pool(**kw):
            cm = tc.tile_pool(**kw)
            pool = cm.__enter__()
            ctx_pools.append((cm, kw["name"]))
            return pool

        def close_pool(pool_name):
            for i, (cm, nm) in enumerate(ctx_pools):
                if nm == pool_name:
                    cm.__exit__(None, None, None)
                    ctx_pools.pop(i)
                    return

        cp = open_pool(name="const", bufs=1)
        dp = open_pool(name="dram", bufs=1, space="DRAM")
        w4 = open_pool(name="wk4", bufs=3)    # f32t [128,512]
        w6 = open_pool(name="wk6", bufs=4)    # f16t [128,512]
        w3 = open_pool(name="wk3", bufs=2)    # rows [1,512] f32
        w2 = open_pool(name="wk2", bufs=2)    # rstdB, onat, rz
        qp = open_pool(name="deq", bufs=2)
        pp = open_pool(name="ps", bufs=2, space="PSUM")
        pro = open_pool(name="pso", bufs=4, space="PSUM")
        prs = open_pool(name="psr", bufs=1, space="PSUM")
        ptr = open_pool(name="pst", bufs=1, space="PSUM")

        # ---- persistent consts in SBUF (~2.6 KB/partition) ----
        shc = cp.tile([128, 1], I32, tag="shc")
        nc.sync.dma_start(out=shc[:], in_=sh_c[:])
        id16 = cp.tile([128, 128], F16, tag="id16")
        nc.sync.dma_start(out=id16[:], in_=id16_c[:])
        ones16 = cp.tile([128, 1], F16, tag="ones16")
        nc.sync.dma_start(out=ones16[:], in_=ones16_c[:])
        onesbf = cp.tile([128, 1], BF16, tag="onesbf")
        nc.sync.dma_start(out=onesbf[:], in_=onesbf_c[:])
        esw16 = cp.tile([128, 63], F16, tag="esw16")
        nc.sync.dma_start(out=esw16[:], in_=esw16_c[:])
        eswbf = cp.tile([128, 63], BF16, tag="eswbf")
        nc.sync.dma_start(out=eswbf[:], in_=eswbf_c[:])
        maskt = cp.tile([128, 896], BF16, tag="maskt")
        nc.sync.dma_start(out=maskt[:], in_=mask_c[:])
        lnw = cp.tile([128, 2 * NKT], F32, tag="lnw")  # ln1 | ln2, [p, kt]
        nc.sync.dma_start(out=lnw[:, 0:NKT],
                          in_=ln1_d[:].rearrange("(kt p) -> p kt", p=128))
        nc.sync.dma_start(out=lnw[:, NKT:2 * NKT],
                          in_=ln2_d[:].rearrange("(kt p) -> p kt", p=128))

        # ---- DRAM scratch ----
        rowsc_d = dp.tile([1, S], F16, tag="rowsc")
        part1_d = dp.tile([HID, S], F32, tag="part1")
        ar1_d = dp.tile([HID, S], F32, tag="ar1", addr_space="Shared")
        h2T_dd = dp.tile([HID, S], F32, tag="h2Td")
        part2_d = dp.tile([HID, S], F32, tag="part2")
        wgu_dd = dp.tile([2 * NIT * 128, NKT * 128], BF16, tag="wgu_dd")
        wdn_dd = dp.tile([NKT * 128, NIT * 128], BF16, tag="wdn_dd")
        rs2_d = dp.tile([OPC, S], F32, tag="rs2")

        def f32t():
            return w4.tile([128, 512], F32, tag="f32t", name="f32t")

        def f16t(dtp=F16):
            return w6.tile([128, 512], dtp, tag="f16t", name="f16t")

        # ============ helper: rms-normalize src -> xT, rsx ============
        def rms_phase(src, lncol, chunks, xsl_fn, sqdt, rsx_fn, esel_t, ones_t):
            for ch in chunks:
                c0, c1 = ch * 512, (ch + 1) * 512
                ssq_ps = prs.tile([32, 512], F32, tag="rs")
                for kt in range(NKT):
                    ht = f32t()
                    nc.gpsimd.dma_start(out=ht[:],
                                        in_=src[kt * 128:(kt + 1) * 128, c0:c1])
                    sq = f16t(sqdt)
                    nc.scalar.activation(sq[:], ht[:], Act.Square)
                    nc.tensor.matmul(ssq_ps[0:1, :], ones_t[:], sq[:],
                                     start=(kt == 0), stop=(kt == NKT - 1))
                trow = w3.tile([1, 512], F32, tag="rows")
                nc.vector.tensor_scalar(out=trow[:], in0=ssq_ps[0:1, :],
                                        scalar1=1.0 / HID, scalar2=EPS,
                                        op0=Alu.mult, op1=Alu.add)
                rrow = w3.tile([1, 512], F32, tag="rows")
                nc.vector.reciprocal(rrow[:], trow[:])
                srow = w3.tile([1, 512], F16, tag="rowsh")
                nc.scalar.activation(srow[:], rrow[:], Act.Sqrt)
                nc.sync.dma_start(out=rowsc_d[0:1, c0:c1], in_=srow[:])
                rstdB = w2.tile([128, 512], F16, tag="rstdB")
                nc.sync.dma_start(
                    out=rstdB[:],
                    in_=rowsc_d[0:1, c0:c1].unsqueeze(1)
                    .broadcast_to([1, 128, 512]))
                rsx_ps = prs.tile([32, 512], F32, tag="rs")
                for kt in range(NKT):
                    ht = f32t()
                    nc.gpsimd.dma_start(out=ht[:],
                                        in_=src[kt * 128:(kt + 1) * 128, c0:c1])
                    xsl = xsl_fn(kt, ch, c0, c1)
                    nc.vector.scalar_tensor_tensor(
                        out=xsl, in0=ht[:],
                        scalar=lnw[:, lncol + kt:lncol + kt + 1],
                        in1=rstdB[:], op0=Alu.mult, op1=Alu.mult)
                    nc.tensor.matmul(rsx_ps[:], esel_t[:, 31 - kt:63 - kt],
                                     xsl, start=(kt == 0), stop=(kt == NKT - 1))
                nc.scalar.activation(rsx_fn(ch, c0, c1), rsx_ps[:], Act.Copy)

        # ============ helper: dequant one [nkt*128, 128] slab ============
        def dequant_slab(qw_ap, sc_ap, ot, nkt, wdt, w16, dve=None):
            ts_eng = dve or nc.vector   # int shift+and (int-only op, any engine)
            dve = nc.vector             # mixed i32*f16 mult: DVE only
            for kt0 in range(0, nkt, 2):
                nsub = min(2, nkt - kt0)
                qwB = qp.tile([128, 2 * 128], I32, tag="qwB")
                nc.scalar.dma_start(out=qwB[:, 0:nsub * 128],
                                    in_=rep_src(qw_ap, ot, kt0, nsub, nkt))
                scB = qp.tile([128, 2 * 128], wdt, tag="scB")
                nc.scalar.dma_start(out=scB[:, 0:nsub * 128],
                                    in_=sc_src(sc_ap, ot, kt0, nsub, nkt))
                for k2 in range(nsub):
                    kt = kt0 + k2
                    nib = qp.tile([128, 128], I32, tag="nib")
                    ts_eng.tensor_scalar(
                        out=nib[:], in0=qwB[:, k2 * 128:(k2 + 1) * 128],
                        scalar1=shc[:], scalar2=15,
                        op0=Alu.logical_shift_right, op1=Alu.bitwise_and)
                    dve.tensor_tensor(
                        out=w16[:, kt * 128:(kt + 1) * 128], in0=nib[:],
                        in1=scB[:, k2 * 128:(k2 + 1) * 128], op=Alu.mult)

        def load_zl(nz_ap, ot, ngr, dtp):
            zl = qp.tile([32, 128], dtp, tag="zl")
            nc.scalar.dma_start(out=zl[0:ngr, :],
                                in_=nz_ap[0:ngr, ot * 128:(ot + 1) * 128])
            return zl

        # MLP weights pre-dequantized to DRAM (bf16), emitted interleaved
        # with the attention phase so dequant DVE/DMA hides under PE work.
        def predeq_unit(u):
            if u < 2 * NIT:
                it = u
                w16 = qp.tile([128, NKT * 128], BF16, tag="w16", name="w16")
                dequant_slab(qw_gu_d[:], sc_gu_d[:], it, NKT, BF16, w16)
                nc.scalar.dma_start(out=wgu_dd[it * 128:(it + 1) * 128, :],
                                    in_=w16[:])
            else:
                ot = u - 2 * NIT
                w16 = qp.tile([128, NKT * 128], BF16, tag="w16", name="w16")
                dequant_slab(qw_dn_d[:], sc_dn_d[:], ot, NIT, BF16, w16)
                nc.scalar.dma_start(out=wdn_dd[ot * 128:(ot + 1) * 128, :],
                                    in_=w16[:, 0:NIT * 128])

        NPRE = 2 * NIT + NKT          # 54 units
        pre_sched = [range(0, 8), range(8, 16), range(16, 24), range(24, 32),
                     range(32, NPRE)]

        # ====================== attention super-phase ======================
        op_ = open_pool(name="oTp", bufs=1)          # outlives x1 pool
        oT = op_.tile([128, NHC * S], F16, tag="oT")
        rso = op_.tile([32, S], F16, tag="rso")

        xp = open_pool(name="xph", bufs=1)
        x1T = xp.tile([128, NKT * S], F16, tag="x1T")
        rsx1 = xp.tile([32, S], F16, tag="rsx1")

        rms_phase(hT_d, 0, range(NCHUNK),
                  lambda kt, ch, c0, c1: x1T[:, kt * S + c0: kt * S + c1],
                  F16, lambda ch, c0, c1: rsx1[:, c0:c1], esw16, ones16)
        if debug:
            for kt in range(NKT):
                for ch in range(NCHUNK):
                    dbt = f32t()
                    nc.scalar.activation(
                        dbt[:], x1T[:, kt * S + ch * 512: kt * S + (ch + 1) * 512],
                        Act.Copy)
                    nc.sync.dma_start(
                        out=dbg["x1T_dbg"][kt * 128:(kt + 1) * 128,
                                           ch * 512:(ch + 1) * 512],
                        in_=dbt[:])

        for h in range(NHC):
            qT = xp.tile([128, S], F16, tag="qT")
            kT = xp.tile([128, S], F16, tag="kT")
            Vn = xp.tile([128, NST * 132], BF16, tag="Vn")
            for which in ("q", "k", "v"):
                ot = {"q": h, "k": NHC + h, "v": 2 * NHC + h}[which]
                dst = {"q": qT, "k": kT, "v": None}[which]
                w16 = qp.tile([128, NKT * 128], F16, tag="w16")
                dequant_slab(qw_qkv_d[:], sc_qkv_d[:], ot, NKT, F16, w16)
                zl = load_zl(nz_qkv_d[:], ot, 32, F16)
                for ch in range(NCHUNK):
                    c0, c1 = ch * 512, (ch + 1) * 512
                    mm = pp.tile([128, 512], F32, tag="mm")
                    for kt in range(NKT):
                        nc.tensor.matmul(
                            mm[:], w16[:, kt * 128:(kt + 1) * 128],
                            x1T[:, kt * S + c0: kt * S + c1],
                            start=(kt == 0), stop=False)
                    nc.tensor.matmul(mm[:], zl[0:32, :], rsx1[:, c0:c1],
                                     start=False, stop=True)
                    if which in ("q", "k"):
                        qsb = f16t()
                        nc.scalar.activation(qsb[:], mm[:], Act.Copy)
                        qsh = f16t()
                        nc.sync.dma_start(out=qsh[0:64, :], in_=qsb[64:128, :])
                        nc.sync.dma_start(out=qsh[64:128, :], in_=qsb[0:64, :])
                        cs = f16t()
                        nc.sync.dma_start(out=cs[:], in_=cosT_d[:, c0:c1])
                        t1 = f16t()
                        nc.vector.tensor_tensor(out=t1[:], in0=qsb[:],
                                                in1=cs[:], op=Alu.mult)
                        sn = f16t()
                        nc.sync.dma_start(out=sn[:], in_=sinfT_d[:, c0:c1])
                        t2 = f16t()
                        nc.vector.tensor_tensor(out=t2[:], in0=qsh[:],
                                                in1=sn[:], op=Alu.mult)
                        nc.vector.tensor_tensor(out=dst[:, c0:c1], in0=t1[:],
                                                in1=t2[:], op=Alu.add)
                    else:
                        vt = f16t()
                        nc.scalar.activation(vt[:], mm[:], Act.Copy)
                        for st4 in range(4):
                            st = ch * 4 + st4
                            trp = ptr.tile([128, 128], F16, tag="tr")
                            nc.tensor.transpose(
                                trp[:], vt[:, st4 * 128:(st4 + 1) * 128],
                                id16[:])
                            nc.scalar.activation(
                                Vn[:, st * 132: st * 132 + 128], trp[:],
                                Act.Copy)
                            nc.vector.memset(
                                Vn[:, st * 132 + 128: st * 132 + 129], 1.0)
            if debug and h == 0:
                for nm, src_t in (("qT_dbg", qT), ("kT_dbg", kT)):
                    for ch in range(NCHUNK):
                        dbt = f32t()
                        nc.scalar.activation(
                            dbt[:], src_t[:, ch * 512:(ch + 1) * 512], Act.Copy)
                        nc.sync.dma_start(
                            out=dbg[nm][:, ch * 512:(ch + 1) * 512], in_=dbt[:])
            # --- attention for this head ---
            for qs in range(NCHUNK):
                oas = [pro.tile([128, 132], F32, tag="oa", name="oa")
                       for _ in range(4)]
                npairs = 4 * qs + 4
                for jj in range(npairs):
                    scp = pp.tile([128, 512], F32, tag="mm")
                    nc.tensor.matmul(scp[:], kT[:, jj * 128:(jj + 1) * 128],
                                     qT[:, qs * 512:(qs + 1) * 512],
                                     start=True, stop=True)
                    ET = f16t(BF16)
                    if jj >= 4 * qs:
                        dd = jj - 4 * qs
                        ms = f32t()
                        nc.vector.scalar_tensor_tensor(
                            out=ms[:], in0=scp[:], scalar=SCALE,
                            in1=maskt[:, 384 - 128 * dd: 896 - 128 * dd],
                            op0=Alu.mult, op1=Alu.add)
                        nc.scalar.activation(ET[:], ms[:], Act.Exp)
                    else:
                        nc.scalar.activation(ET[:], scp[:], Act.Exp,
                                             scale=SCALE)
                    for qt in range(4):
                        nc.tensor.matmul(
                            oas[qt][:, 0:129],
                            ET[:, qt * 128:(qt + 1) * 128],
                            Vn[:, jj * 132: jj * 132 + 129],
                            start=(jj == 0), stop=(jj == npairs - 1))
                for qt in range(4):
                    oa = oas[qt]
                    rz = w2.tile([128, 1], F32, tag="rz")
                    nc.vector.reciprocal(rz[:], oa[:, 128:129])
                    onat = w2.tile([128, 128], F16, tag="onat")
                    nc.vector.tensor_scalar(out=onat[:],
                                            in0=oa[:, 0:128],
                                            scalar1=rz[:], scalar2=None,
                                            op0=Alu.mult)
                    trp = ptr.tile([128, 128], F16, tag="tr")
                    nc.tensor.transpose(trp[:], onat[:], id16[:])
                    st = qs * 4 + qt
                    nc.scalar.activation(
                        oT[:, h * S + st * 128: h * S + (st + 1) * 128],
                        trp[:], Act.Copy)
            for u in pre_sched[h]:
                predeq_unit(u)
        close_pool("xph")
        for u in pre_sched[4]:
            predeq_unit(u)

        if debug:
            for hh in range(NHC):
                for ch in range(NCHUNK):
                    dbt = f32t()
                    nc.scalar.activation(
                        dbt[:], oT[:, hh * S + ch * 512: hh * S + (ch + 1) * 512],
                        Act.Copy)
                    nc.sync.dma_start(
                        out=dbg["oT_dbg"][hh * 128:(hh + 1) * 128,
                                          ch * 512:(ch + 1) * 512],
                        in_=dbt[:])

        # ---- rowsums of oT (4 groups) ----
        for ch in range(NCHUNK):
            c0, c1 = ch * 512, (ch + 1) * 512
            rs_ps = prs.tile([32, 512], F32, tag="rs")
            for kt in range(NHC):
                nc.tensor.matmul(rs_ps[:], esw16[:, 31 - kt:63 - kt],
                                 oT[:, kt * S + c0: kt * S + c1],
                                 start=(kt == 0), stop=(kt == NHC - 1))
            nc.scalar.activation(rso[:, c0:c1], rs_ps[:], Act.Copy)

        # ---- o-projection partials -> part1_d ----
        for ot in range(NKT):
            w16 = qp.tile([128, NKT * 128], F16, tag="w16")
            dequant_slab(qw_o_d[:], sc_o_d[:], ot, NHC, F16, w16)
            zl = load_zl(nz_o_d[:], ot, NHC, F16)
            for ch in range(NCHUNK):
                c0, c1 = ch * 512, (ch + 1) * 512
                mm = pp.tile([128, 512], F32, tag="mm")
                for kt in range(NHC):
                    nc.tensor.matmul(
                        mm[:], w16[:, kt * 128:(kt + 1) * 128],
                        oT[:, kt * S + c0: kt * S + c1],
                        start=(kt == 0), stop=False)
                nc.tensor.matmul(mm[:], zl[0:NHC, :], rso[0:NHC, c0:c1],
                                 start=False, stop=True)
                pt = f32t()
                nc.scalar.activation(pt[:], mm[:], Act.Copy)
                nc.sync.dma_start(
                    out=part1_d[ot * 128:(ot + 1) * 128, c0:c1], in_=pt[:])
        close_pool("oTp")

        # ====================== exchange 1: AllReduce =======================
        nc.gpsimd.collective_compute(
            "AllReduce", Alu.add,
            replica_groups=[list(range(NCORES))],
            ins=[part1_d[:].opt()], outs=[ar1_d[:].opt()])
        if debug:
            nc.sync.dma_start(out=dbg["p1_dbg"][:], in_=ar1_d[:])

        # ---- h2T = hT + ar1 -> DRAM ----
        for ot in range(NKT):
            for ch in range(NCHUNK):
                c0, c1 = ch * 512, (ch + 1) * 512
                ta = f32t()
                nc.gpsimd.dma_start(out=ta[:],
                                    in_=ar1_d[ot * 128:(ot + 1) * 128, c0:c1])
                th = f32t()
                nc.gpsimd.dma_start(out=th[:],
                                    in_=hT_d[ot * 128:(ot + 1) * 128, c0:c1])
                t2 = f32t()
                nc.vector.tensor_tensor(out=t2[:], in0=ta[:], in1=th[:],
                                        op=Alu.add)
                nc.gpsimd.dma_start(
                    out=h2T_dd[ot * 128:(ot + 1) * 128, c0:c1], in_=t2[:])

        # ====================== MLP super-phase =============================
        # (weights were pre-dequantized to DRAM, interleaved with attention)
        xp2 = open_pool(name="xph2", bufs=1)
        x2Tc = xp2.tile([128, NKT * 512], BF16, tag="x2Tc")
        yTc = xp2.tile([128, NIT * 512], BF16, tag="yTc")
        rsx2c = xp2.tile([32, 512], BF16, tag="rsx2c")
        rsyc = xp2.tile([32, 512], BF16, tag="rsyc")

        for ch in range(NCHUNK):
            c0, c1 = ch * 512, (ch + 1) * 512
            rms_phase(h2T_dd, NKT, [ch],
                      lambda kt, _c, _0, _1: x2Tc[:, kt * 512:(kt + 1) * 512],
                      BF16, lambda _c, _0, _1: rsx2c[:], eswbf, onesbf)
            if debug:
                for kt in range(NKT):
                    dbt = f32t()
                    nc.scalar.activation(dbt[:], x2Tc[:, kt * 512:(kt + 1) * 512],
                                         Act.Copy)
                    nc.sync.dma_start(
                        out=dbg["x2T_dbg"][kt * 128:(kt + 1) * 128, c0:c1],
                        in_=dbt[:])
            # gate/up -> yTc
            for it in range(NIT):
                wg = qp.tile([128, NKT * 128], BF16, tag="w16")
                nc.scalar.dma_start(out=wg[:],
                                    in_=wgu_dd[it * 128:(it + 1) * 128, :])
                zlg = load_zl(nz_gu_d[:], it, 32, BF16)
                wu = qp.tile([128, NKT * 128], BF16, tag="w16")
                nc.scalar.dma_start(
                    out=wu[:],
                    in_=wgu_dd[(NIT + it) * 128:(NIT + it + 1) * 128, :])
                zlu = load_zl(nz_gu_d[:], NIT + it, 32, BF16)
                gp = pp.tile([128, 512], F32, tag="mm")
                for kt in range(NKT):
                    nc.tensor.matmul(
                        gp[:], wg[:, kt * 128:(kt + 1) * 128],
                        x2Tc[:, kt * 512:(kt + 1) * 512],
                        start=(kt == 0), stop=False)
                nc.tensor.matmul(gp[:], zlg[0:32, :], rsx2c[:],
                                 start=False, stop=True)
                up = pp.tile([128, 512], F32, tag="mm")
                for kt in range(NKT):
                    nc.tensor.matmul(
                        up[:], wu[:, kt * 128:(kt + 1) * 128],
                        x2Tc[:, kt * 512:(kt + 1) * 512],
                        start=(kt == 0), stop=False)
                nc.tensor.matmul(up[:], zlu[0:32, :], rsx2c[:],
                                 start=False, stop=True)
                sg = f32t()
                nc.scalar.activation(sg[:], gp[:], Act.Silu)
                nc.vector.tensor_tensor(
                    out=yTc[:, it * 512:(it + 1) * 512],
                    in0=sg[:], in1=up[:], op=Alu.mult)
            if debug:
                for kt in range(NIT):
                    dbt = f32t()
                    nc.scalar.activation(dbt[:], yTc[:, kt * 512:(kt + 1) * 512],
                                         Act.Copy)
                    nc.sync.dma_start(
                        out=dbg["yT_dbg"][kt * 128:(kt + 1) * 128, c0:c1],
                        in_=dbt[:])
            # rowsums of yTc
            rs_ps = prs.tile([32, 512], F32, tag="rs")
            for kt in range(NIT):
                nc.tensor.matmul(rs_ps[:], eswbf[:, 31 - kt:63 - kt],
                                 yTc[:, kt * 512:(kt + 1) * 512],
                                 start=(kt == 0), stop=(kt == NIT - 1))
            nc.scalar.activation(rsyc[:], rs_ps[:], Act.Copy)
            # down partials + h2/8 -> part2_d
            for ot in range(NKT):
                w16 = qp.tile([128, NKT * 128], BF16, tag="w16")
                nc.scalar.dma_start(out=w16[:, 0:NIT * 128],
                                    in_=wdn_dd[ot * 128:(ot + 1) * 128, :])
                zl = load_zl(nz_dn_d[:], ot, NIT, BF16)
                mm = pp.tile([128, 512], F32, tag="mm")
                for kt in range(NIT):
                    nc.tensor.matmul(
                        mm[:], w16[:, kt * 128:(kt + 1) * 128],
                        yTc[:, kt * 512:(kt + 1) * 512],
                        start=(kt == 0), stop=False)
                nc.tensor.matmul(mm[:], zl[0:NIT, :], rsyc[0:NIT, :],
                                 start=False, stop=True)
                th = f32t()
                nc.gpsimd.dma_start(out=th[:],
                                    in_=h2T_dd[ot * 128:(ot + 1) * 128, c0:c1])
                pt = f32t()
                nc.vector.scalar_tensor_tensor(
                    out=pt[:], in0=th[:], scalar=1.0 / NCORES, in1=mm[:],
                    op0=Alu.mult, op1=Alu.add)
                nc.sync.dma_start(
                    out=part2_d[ot * 128:(ot + 1) * 128, c0:c1], in_=pt[:])
        close_pool("xph2")

        # ====================== exchange 2: ReduceScatter ===================
        nc.gpsimd.collective_compute(
            "ReduceScatter", Alu.add,
            replica_groups=[list(range(NCORES))],
            ins=[part2_d[:].opt()], outs=[rs2_d[:].opt()])
        nc.sync.dma_start(out=outT_d[:], in_=rs2_d[:])

        for cm, nm in reversed(ctx_pools):
            cm.__exit__(None, None, None)
        ctx_pools.clear()

    nc.compile()
    return nc


def _host_prep(inputs):
    """Build the 8 per-core input maps from full inputs."""
    import ml_dtypes
    bf16 = ml_dtypes.bfloat16
    f16 = np.float16

    def unpack_z1(qz):
        sh = (np.arange(8, dtype=np.uint32) * 4)
        z = ((qz[:, :, None].view(np.uint32) >> sh[None, None, :]) & 15)
        return z.reshape(qz.shape[0], -1).astype(np.float32) + 1.0

    h = np.asarray(inputs["hidden_states"], np.float32)[0]     # [S, HID]
    hT = np.ascontiguousarray(h.T)                             # [HID, S]
    sin = np.asarray(inputs["sin"], np.float32)                # [S, HD]
    cos = np.asarray(inputs["cos"], np.float32)
    cosT = np.ascontiguousarray(cos.T).astype(f16)
    sinf = sin.T.copy()
    sinf[0:64, :] *= -1.0                                      # rot-half sign fold
    sinfT = np.ascontiguousarray(sinf).astype(f16)

    qkv_qw, qkv_sc, qkv_nz = [], [], []
    for nm in ("q", "k", "v"):
        qw = np.asarray(inputs["qw_" + nm])
        sc = np.asarray(inputs["sc_" + nm], np.float32)
        z1 = unpack_z1(np.asarray(inputs["qz_" + nm]))
        qkv_qw.append(qw); qkv_sc.append(sc); qkv_nz.append(-(z1 * sc))

    qw_o = np.asarray(inputs["qw_o"])
    sc_o = np.asarray(inputs["sc_o"], np.float32)
    nz_o = -(unpack_z1(np.asarray(inputs["qz_o"])) * sc_o)

    def pad_cols(a, w):
        out = np.zeros((a.shape[0], w), a.dtype)
        out[:, :a.shape[1]] = a
        return out

    qw_g = pad_cols(np.asarray(inputs["qw_gate"]), IPAD)
    qw_u = pad_cols(np.asarray(inputs["qw_up"]), IPAD)
    sc_g = pad_cols(np.asarray(inputs["sc_gate"], np.float32), IPAD)
    sc_u = pad_cols(np.asarray(inputs["sc_up"], np.float32), IPAD)
    nz_g = pad_cols(-(unpack_z1(np.asarray(inputs["qz_gate"]))
                      * np.asarray(inputs["sc_gate"], np.float32)), IPAD)
    nz_u = pad_cols(-(unpack_z1(np.asarray(inputs["qz_up"]))
                      * np.asarray(inputs["sc_up"], np.float32)), IPAD)

    qw_dn = np.zeros((IPAD // 8, HID), np.int32)
    qw_dn[:INTER // 8] = np.asarray(inputs["qw_down"])
    sc_dn = np.zeros((IPAD // GS, HID), np.float32)
    sc_dn[:INTER // GS] = np.asarray(inputs["sc_down"], np.float32)
    nz_dn = np.zeros((IPAD // GS, HID), np.float32)
    nz_dn[:INTER // GS] = -(unpack_z1(np.asarray(inputs["qz_down"]))
                            * np.asarray(inputs["sc_down"], np.float32))

    ln1 = np.asarray(inputs["ln1_w"], np.float32)
    ln2 = np.asarray(inputs["ln2_w"], np.float32)

    def repack_qw(qw):
        # [nkt*16, notile*128] -> [notile*16, nkt*128], rows o*16+r,
        # k-tiles contiguous per out-tile
        nkt = qw.shape[0] // 16
        notile = qw.shape[1] // 128
        return np.ascontiguousarray(
            qw.reshape(nkt, 16, notile, 128).transpose(2, 1, 0, 3)
            .reshape(notile * 16, nkt * 128))

    def repack_sc(sc):
        # [G, notile*128] -> [notile*G, 128], rows o*G+g
        G = sc.shape[0]
        notile = sc.shape[1] // 128
        return np.ascontiguousarray(
            sc.reshape(G, notile, 128).transpose(1, 0, 2)
            .reshape(notile * G, 128))

    maps = []
    for c in range(NCORES):
        cs = slice(c * OPC, (c + 1) * OPC)
        isl = slice(c * IPC, (c + 1) * IPC)
        m = {
            "hT": hT, "cosT": cosT, "sinfT": sinfT, "ln1": ln1, "ln2": ln2,
            "qw_qkv": repack_qw(
                np.concatenate([qkv_qw[i][:, cs] for i in range(3)], axis=1)),
            "sc_qkv": repack_sc(
                np.concatenate([qkv_sc[i][:, cs] for i in range(3)],
                               axis=1).astype(f16)),
            "nz_qkv": np.ascontiguousarray(
                np.concatenate([qkv_nz[i][:, cs] for i in range(3)],
                               axis=1)).astype(f16),
            "qw_o": repack_qw(qw_o[c * OPC // 8:(c + 1) * OPC // 8]),
            "sc_o": repack_sc(
                sc_o[c * OPC // GS:(c + 1) * OPC // GS].astype(f16)),
            "nz_o": np.ascontiguousarray(
                nz_o[c * OPC // GS:(c + 1) * OPC // GS]).astype(f16),
            "qw_gu": repack_qw(
                np.concatenate([qw_g[:, isl], qw_u[:, isl]], axis=1)),
            "sc_gu": repack_sc(
                np.concatenate([sc_g[:, isl], sc_u[:, isl]],
                               axis=1).astype(bf16)),
            "nz_gu": np.ascontiguousarray(
                np.concatenate([nz_g[:, isl], nz_u[:, isl]],
                               axis=1)).astype(bf16),
            "qw_dn": repack_qw(qw_dn[c * IPC // 8:(c + 1) * IPC // 8]),
            "sc_dn": repack_sc(
                sc_dn[c * NIT:(c + 1) * NIT].astype(bf16)),
            "nz_dn": np.ascontiguousarray(
                nz_dn[c * NIT:(c + 1) * NIT]).astype(bf16),
        }
        maps.append(m)
    return maps


def run(inputs, debug=False, trace=False):
    from concourse.bass_utils import run_bass_kernel_spmd
    key = ("dbg" if debug else "rel")
    if key not in _BUILD_CACHE:
        _BUILD_CACHE[key] = _build(debug=debug)
    nc = _BUILD_CACHE[key]
    maps = _host_prep(inputs)
    res = run_bass_kernel_spmd(nc, maps, core_ids=list(range(NCORES)),
                               trace=trace)
    outT = np.concatenate([res.results[c]["outT"] for c in range(NCORES)],
                          axis=0)
    out = np.ascontiguousarray(outT.T)[None]
    return out, res


def kernel(**inputs):
    out, _ = run(inputs)
    return out



# revision 14
# speedup vs baseline: 2.0569x; 2.0569x over previous
"""ExLlama transformer layer (GPTQ int4) on 8 TRN2 NeuronCores, tensor-parallel.

Self-contained: hardcodes shapes from the problem spec.
  B=1, S=2048, HID=4096, INTER=11008, HEADS=32, HD=128, GS=128.

v2 design (vs baseline): JIT SBUF dequant everywhere (no DRAM weight
round-trips), bf16 row-group-chunked ReduceScatters overlapped with
o-proj/down, sequence(feature)-sharded residual + RMS2 with tiny stats
AllReduce + bf16 AllGather, swapped PV matmul (oT direct in [d,s]
layout, no output transposes), score/exp software pipelining.

Feature ownership: reduce-scatter over row-groups hands core c the
original feature blocks {r*1024 + c*128 : r in 0..3} ("AG order").
The host permutes gate/up k-rows to AG order, supplies h-slice /
ln2-slice in AG order, and re-scatters the final output pieces.
"""
import sys

sys.path.insert(0, "/opt/trn_rl_repo")

import numpy as np

S = 2048
HID = 4096
HD = 128
GS = 128
INTER = 11008
NCORES = 8
IPC = 1408                      # padded inter features per core
IPAD = IPC * NCORES             # 11264
NKT = HID // 128                # 32 k-tiles over HID
NIT = IPC // 128                # 11 k-tiles over per-core inter
OPC = HID // NCORES             # 512 own features per core, 4 blocks
NHC = OPC // HD                 # 4 heads / 4 row-groups per core
CHUNK = 512
NCHUNK = S // CHUNK             # 4
SCALE = 1.0 / float(np.sqrt(HD))
EPS = 1e-6

# AG-order block permutation: AG output row-block b (core b//4, rg b%4)
# holds original feature block (b % 4) * 8 + b // 4
AGPERM = [(b % 4) * 8 + b // 4 for b in range(32)]

_BUILD_CACHE = {}


def _build(debug=False):
    import concourse.bacc as bacc
    import concourse.mybir as mybir
    import concourse.tile as tile
    import ml_dtypes

    dt = mybir.dt
    F32, F16, BF16, I32 = dt.float32, dt.float16, dt.bfloat16, dt.int32
    Alu = mybir.AluOpType
    Act = mybir.ActivationFunctionType

    nc = bacc.Bacc("TRN2", target_bir_lowering=False, num_devices=NCORES)

    # ---------------- external I/O ----------------
    hb_d = nc.dram_tensor("hb", [HID, S], BF16, kind="ExternalInput")
    hsl_d = nc.dram_tensor("hsl", [OPC, S], F32, kind="ExternalInput")
    cosT_d = nc.dram_tensor("cosT", [HD, S], F16, kind="ExternalInput")
    sinfT_d = nc.dram_tensor("sinfT", [HD, S], F16, kind="ExternalInput")
    ln1_d = nc.dram_tensor("ln1", [HID], F32, kind="ExternalInput")
    ln2p_d = nc.dram_tensor("ln2p", [OPC], F32, kind="ExternalInput")
    hmask_d = nc.dram_tensor("hmask", [128, NKT], F32, kind="ExternalInput")
    qw_qkv_d = nc.dram_tensor("qw_qkv", [12 * 16, NKT * 128], I32, kind="ExternalInput")
    sc_qkv_d = nc.dram_tensor("sc_qkv", [12 * NKT, 128], F16, kind="ExternalInput")
    nz_qkv_d = nc.dram_tensor("nz_qkv", [32, 3 * OPC], F16, kind="ExternalInput")
    qw_o_d = nc.dram_tensor("qw_o", [NKT * 16, NHC * 128], I32, kind="ExternalInput")
    sc_o_d = nc.dram_tensor("sc_o", [NKT * NHC, 128], F16, kind="ExternalInput")
    nz_o_d = nc.dram_tensor("nz_o", [OPC // GS, HID], F16, kind="ExternalInput")
    qw_gu_d = nc.dram_tensor("qw_gu", [2 * NIT * 16, NKT * 128], I32, kind="ExternalInput")
    sc_gu_d = nc.dram_tensor("sc_gu", [2 * NIT * NKT, 128], BF16, kind="ExternalInput")
    nz_gu_d = nc.dram_tensor("nz_gu", [32, 2 * IPC], BF16, kind="ExternalInput")
    qw_dn_d = nc.dram_tensor("qw_dn", [NKT * 16, NIT * 128], I32, kind="ExternalInput")
    sc_dn_d = nc.dram_tensor("sc_dn", [NKT * NIT, 128], BF16, kind="ExternalInput")
    nz_dn_d = nc.dram_tensor("nz_dn", [NIT, HID], BF16, kind="ExternalInput")

    outT_d = nc.dram_tensor("outT", [OPC, S], F32, kind="ExternalOutput")
    dbg = {}
    if debug:
        dbg["rowsc_dbg"] = nc.dram_tensor("rowsc_dbg", [1, 6 * S], F16, kind="ExternalOutput")
        dbg["x1T_dbg"] = nc.dram_tensor("x1T_dbg", [2 * 128, S], F32, kind="ExternalOutput")
        dbg["qT_dbg"] = nc.dram_tensor("qT_dbg", [128, S], F16, kind="ExternalOutput")
        dbg["kT_dbg"] = nc.dram_tensor("kT_dbg", [128, S], F16, kind="ExternalOutput")
        dbg["oT_dbg"] = nc.dram_tensor("oT_dbg", [NHC * 128, S], F16, kind="ExternalOutput")
        dbg["h2s_dbg"] = nc.dram_tensor("h2s_dbg", [NHC * 128, S], F32, kind="ExternalOutput")
        dbg["x2own_dbg"] = nc.dram_tensor("x2own_dbg", [OPC, S], BF16, kind="ExternalOutput")
        dbg["rsy_dbg"] = nc.dram_tensor("rsy_dbg", [NIT, S], BF16, kind="ExternalOutput")
        dbg["rsx1_dbg"] = nc.dram_tensor("rsx1_dbg", [32, S], F16, kind="ExternalOutput")

    # ---------------- inline constants ----------------
    p = np.arange(128)
    sh_c = nc.inline_tensor(((p % 8) * 4).astype(np.int32)[:, None], name="shc")
    id16_c = nc.inline_tensor(np.eye(128, dtype=np.float16), name="id16")
    ones16_c = nc.inline_tensor(np.ones((128, 1), np.float16), name="ones16")
    onesbf_c = nc.inline_tensor(np.ones((128, 1), ml_dtypes.bfloat16),
                                name="onesbf")
    eslw = np.zeros((128, 63), np.float16)
    eslw[:, 31] = 1.0
    esw16_c = nc.inline_tensor(eslw, name="esw16")
    eswbf_c = nc.inline_tensor(eslw.astype(ml_dtypes.bfloat16), name="eswbf")
    # wide causal 0/1 multiplicative mask, S^T layout: mkw[k, j] with
    # qq = j-384: 1 if qq >= k else 0.  Slice [384-128*dd : 896-128*dd].
    j = np.arange(896)
    mkw = np.where((j[None, :] - 384) >= p[:, None], 1.0, 0.0)
    mask_c = nc.inline_tensor(mkw.astype(ml_dtypes.bfloat16), name="maskc")

    def rep_src(qw_ap, ot, kt0, nsub, nkt):
        sl = qw_ap[ot * 16:(ot + 1) * 16,
                   kt0 * 128:(kt0 + nsub) * 128]
        return sl.unsqueeze(1).broadcast_to([16, 8, nsub * 128])

    def sc_src(sc_ap, ot, kt0, nsub, nkt):
        sl = sc_ap[ot * nkt + kt0: ot * nkt + kt0 + nsub, :]
        return sl.rearrange("g c -> (g c)").unsqueeze(0).unsqueeze(0) \
                 .broadcast_to([1, 128, nsub * 128])

    with tile.TileContext(nc) as tc:
        ctx_pools = []

        def open_pool(**kw):
            cm = tc.tile_pool(**kw)
            pool = cm.__enter__()
            ctx_pools.append((cm, kw["name"]))
            return pool

        def close_pool(pool_name):
            for i, (cm, nm) in enumerate(ctx_pools):
                if nm == pool_name:
                    cm.__exit__(None, None, None)
                    ctx_pools.pop(i)
                    return

        cp = open_pool(name="const", bufs=1)
        dp = open_pool(name="dram", bufs=1, space="DRAM")
        w4 = open_pool(name="wk4", bufs=4)    # f32t [128,512]
        w6 = open_pool(name="wk6", bufs=6)    # f16t [128,512]
        w3 = open_pool(name="wk3", bufs=2)    # rows [1,512]
        w2 = open_pool(name="wk2", bufs=2)    # broadcast tiles [128,512] f16
        qp = open_pool(name="deq", bufs=2)    # qwB/scB/nib [128,512]
        wsl = open_pool(name="wslab", bufs=2)  # dequantized weight slabs
        pp = open_pool(name="ps", bufs=2, space="PSUM")    # proj mm + scores
        pv = open_pool(name="psv", bufs=2, space="PSUM")   # pv accum / up
        pz = open_pool(name="psz", bufs=2, space="PSUM")   # [1,512] rows
        prs = open_pool(name="psr", bufs=1, space="PSUM")  # esel rowsums
        ptr = open_pool(name="pst", bufs=1, space="PSUM")  # transposes

        # ---- persistent consts in SBUF ----
        shc = cp.tile([128, 1], I32, tag="shc")
        nc.sync.dma_start(out=shc[:], in_=sh_c[:])
        id16 = cp.tile([128, 128], F16, tag="id16")
        nc.sync.dma_start(out=id16[:], in_=id16_c[:])
        ones16 = cp.tile([128, 1], F16, tag="ones16")
        nc.sync.dma_start(out=ones16[:], in_=ones16_c[:])
        onesbf = cp.tile([128, 1], BF16, tag="onesbf")
        nc.sync.dma_start(out=onesbf[:], in_=onesbf_c[:])
        esw16 = cp.tile([128, 63], F16, tag="esw16")
        nc.sync.dma_start(out=esw16[:], in_=esw16_c[:])
        eswbf = cp.tile([128, 63], BF16, tag="eswbf")
        nc.sync.dma_start(out=eswbf[:], in_=eswbf_c[:])
        lnw = cp.tile([128, NKT], F32, tag="lnw")
        nc.sync.dma_start(out=lnw[:],
                          in_=ln1_d[:].rearrange("(kt p) -> p kt", p=128))
        ln2s = cp.tile([128, NHC], F32, tag="ln2s")
        nc.sync.dma_start(out=ln2s[:],
                          in_=ln2p_d[:].rearrange("(rg p) -> p rg", p=128))
        hmask = cp.tile([128, NKT], F32, tag="hmask")
        nc.sync.dma_start(out=hmask[:], in_=hmask_d[:])

        # ---- DRAM scratch ----
        ssq1p_d = dp.tile([1, S], F32, tag="ssq1p")
        ssq1s_d = dp.tile([1, S], F32, tag="ssq1s", addr_space="Shared")
        ssq2p_d = dp.tile([1, S], F32, tag="ssq2p")
        ssq2s_d = dp.tile([1, S], F32, tag="ssq2s", addr_space="Shared")
        rowsc_d = dp.tile([1, 6 * S], F16, tag="rowsc")   # rstd1 | (unused) | rstd2
        rz_d = dp.tile([1, NHC * S], F32, tag="rzd")
        part1_d = [dp.tile([HID, CHUNK], BF16, tag=f"part1_{ch}", name=f"part1_{ch}")
                   for ch in range(NCHUNK)]
        rs1o_d = [dp.tile([OPC, CHUNK], BF16, tag=f"rs1o_{ch}", name=f"rs1o_{ch}")
                  for ch in range(NCHUNK)]
        oT_d = dp.tile([NHC * 128, S], F16, tag="oTd")
        x2own_d = dp.tile([OPC, S], BF16, tag="x2own")
        x2all_d = dp.tile([HID, S], BF16, tag="x2all", addr_space="Shared")
        yT_d = dp.tile([NIT * 128, S], BF16, tag="yTd")
        part2_d = [dp.tile([8 * 128, S], BF16, tag=f"part2_{r}", name=f"part2_{r}")
                   for r in range(4)]
        rs2o_d = [dp.tile([128, S], BF16, tag=f"rs2o_{r}", name=f"rs2o_{r}")
                  for r in range(4)]

        def f32t():
            return w4.tile([128, 512], F32, tag="f32t", name="f32t")

        def f16t(dtp=F16):
            return w6.tile([128, 512], dtp, tag="f16t", name="f16t")

        # ============ helper: dequant one [nkt*128, 128] slab ============
        # FD=512 batches: gpsimd does int shift+and, DVE the i32*f16 mult.
        def dequant_slab(qw_ap, sc_ap, ot, nkt, wdt, w16):
            for kt0 in range(0, nkt, 4):
                nsub = min(4, nkt - kt0)
                fd = nsub * 128
                qwB = qp.tile([128, 512], I32, tag="qwB", name="qwB")
                nc.sync.dma_start(out=qwB[:, 0:fd],
                                  in_=rep_src(qw_ap, ot, kt0, nsub, nkt))
                scB = qp.tile([128, 512], wdt, tag="scB", name="scB")
                nc.scalar.dma_start(out=scB[:, 0:fd],
                                    in_=sc_src(sc_ap, ot, kt0, nsub, nkt))
                nib = qp.tile([128, 512], I32, tag="nib", name="nib")
                nc.vector.tensor_scalar(
                    out=nib[:, 0:fd], in0=qwB[:, 0:fd],
                    scalar1=shc[:], scalar2=15,
                    op0=Alu.logical_shift_right, op1=Alu.bitwise_and)
                nc.vector.tensor_tensor(
                    out=w16[:, kt0 * 128:kt0 * 128 + fd], in0=nib[:, 0:fd],
                    in1=scB[:, 0:fd], op=Alu.mult)

        def load_zl(nz_ap, ot, ngr, dtp):
            zl = qp.tile([32, 128], dtp, tag="zl", name="zl")
            nc.scalar.dma_start(out=zl[0:ngr, :],
                                in_=nz_ap[0:ngr, ot * 128:(ot + 1) * 128])
            return zl

        def rstd_from(ssq_s_ap, col_off):
            # ssq partial-sum row -> 1/sqrt(mean+eps) f16 row in rowsc_d
            for ch in range(NCHUNK):
                c0, c1 = ch * 512, (ch + 1) * 512
                trow = w3.tile([1, 512], F32, tag="rows", name="trow")
                nc.gpsimd.dma_start(out=trow[:], in_=ssq_s_ap[0:1, c0:c1])
                t2 = w3.tile([1, 512], F32, tag="rows", name="t2row")
                nc.vector.tensor_scalar(out=t2[:], in0=trow[:],
                                        scalar1=1.0 / HID, scalar2=EPS,
                                        op0=Alu.mult, op1=Alu.add)
                rrow = w3.tile([1, 512], F32, tag="rows", name="rrow")
                nc.vector.reciprocal(rrow[:], t2[:])
                srow = w3.tile([1, 512], F16, tag="rowsh", name="srow")
                nc.scalar.activation(srow[:], rrow[:], Act.Sqrt)
                nc.sync.dma_start(out=rowsc_d[0:1, col_off + c0:col_off + c1],
                                  in_=srow[:])

        def bcast(col_off, c0, c1, name):
            bt = w2.tile([128, 512], F16, tag="bc", name=name)
            nc.scalar.dma_start(
                out=bt[:],
                in_=rowsc_d[0:1, col_off + c0:col_off + c1].unsqueeze(1)
                .broadcast_to([1, 128, 512]))
            return bt

        # =================== P0: rms1 (sharded stats) ===================
        xp = open_pool(name="xph", bufs=1)
        x1T = xp.tile([128, NKT * S], F16, tag="x1T")
        rsx1 = xp.tile([32, S], F16, tag="rsx1")
        maskt = xp.tile([128, 896], BF16, tag="maskt")
        nc.sync.dma_start(out=maskt[:], in_=mask_c[:])
        cosS = xp.tile([128, S], F16, tag="cosS")
        nc.scalar.dma_start(out=cosS[:], in_=cosT_d[:])
        sinS = xp.tile([128, S], F16, tag="sinS")
        nc.scalar.dma_start(out=sinS[:], in_=sinfT_d[:])

        for ch in range(NCHUNK):
            c0, c1 = ch * 512, (ch + 1) * 512
            ssq_ps = pz.tile([1, 512], F32, tag="zrow", name="ssq1")
            for rg in range(NHC):
                ht = f32t()
                nc.gpsimd.dma_start(out=ht[:],
                                    in_=hsl_d[rg * 128:(rg + 1) * 128, c0:c1])
                sq = f16t()
                nc.scalar.activation(sq[:], ht[:], Act.Square)
                nc.tensor.matmul(ssq_ps[:], ones16[:], sq[:],
                                 start=(rg == 0), stop=(rg == NHC - 1))
            srow = w3.tile([1, 512], F32, tag="rows", name="sr1")
            nc.scalar.activation(srow[:], ssq_ps[:], Act.Copy)
            nc.sync.dma_start(out=ssq1p_d[0:1, c0:c1], in_=srow[:])
        nc.gpsimd.collective_compute(
            "AllReduce", Alu.add, replica_groups=[list(range(NCORES))],
            ins=[ssq1p_d[:].opt()], outs=[ssq1s_d[:].opt()])
        rstd_from(ssq1s_d, 0)

        # normalize full h -> x1T (f16), and rsx1 esel rowsums
        for ch in range(NCHUNK):
            c0, c1 = ch * 512, (ch + 1) * 512
            rstdB = bcast(0, c0, c1, "rstdB1")
            rsx_ps = prs.tile([32, 512], F32, tag="rs", name="rsx1p")
            for kt in range(NKT):
                hbt = f16t(BF16)
                nc.gpsimd.dma_start(out=hbt[:],
                                    in_=hb_d[kt * 128:(kt + 1) * 128, c0:c1])
                xsl = x1T[:, kt * S + c0: kt * S + c1]
                nc.vector.scalar_tensor_tensor(
                    out=xsl, in0=hbt[:],
                    scalar=lnw[:, kt:kt + 1],
                    in1=rstdB[:], op0=Alu.mult, op1=Alu.mult)
                nc.tensor.matmul(rsx_ps[:], esw16[:, 31 - kt:63 - kt],
                                 xsl, start=(kt == 0), stop=(kt == NKT - 1))
            nc.scalar.activation(rsx1[:, c0:c1], rsx_ps[:], Act.Copy)

        if debug:
            for kt in range(2):
                for ch in range(NCHUNK):
                    dbt = f32t()
                    nc.vector.tensor_scalar(
                        out=dbt[:], in0=x1T[:, kt * S + ch * 512: kt * S + (ch + 1) * 512],
                        scalar1=1.0, scalar2=None, op0=Alu.mult)
                    nc.sync.dma_start(
                        out=dbg["x1T_dbg"][kt * 128:(kt + 1) * 128,
                                           ch * 512:(ch + 1) * 512], in_=dbt[:])
            nc.sync.dma_start(out=dbg["rsx1_dbg"][:], in_=rsx1[:])
        # =================== P1: qkv + attention per head ===================
        for h in range(NHC):
            qT = xp.tile([128, S], F16, tag="qT", name="qT")
            kT = xp.tile([128, S], F16, tag="kT", name="kT")
            Vn = xp.tile([128, 16 * 128], BF16, tag="Vn", name="Vn")
            vt_pend = []            # deferred V transposes: (vt, ch)
            for which in ("q", "k", "v"):
                ot = {"q": h, "k": NHC + h, "v": 2 * NHC + h}[which]
                dst = {"q": qT, "k": kT, "v": None}[which]
                w16 = wsl.tile([128, NKT * 128], F16, tag="w16", name="w16")
                dequant_slab(qw_qkv_d[:], sc_qkv_d[:], ot, NKT, F16, w16)
                zl = load_zl(nz_qkv_d[:], ot, 32, F16)
                for ch in range(NCHUNK):
                    c0, c1 = ch * 512, (ch + 1) * 512
                    mm = pp.tile([128, 512], F32, tag="mm", name="mmqkv")
                    for kt in range(NKT):
                        nc.tensor.matmul(
                            mm[:], w16[:, kt * 128:(kt + 1) * 128],
                            x1T[:, kt * S + c0: kt * S + c1],
                            start=(kt == 0), stop=False)
                    nc.tensor.matmul(mm[:], zl[0:32, :], rsx1[:, c0:c1],
                                     start=False, stop=True)
                    if which in ("q", "k"):
                        qsb = f16t()
                        nc.scalar.activation(qsb[:], mm[:], Act.Copy)
                        qsh = f16t()
                        nc.sync.dma_start(out=qsh[0:64, :], in_=qsb[64:128, :])
                        nc.sync.dma_start(out=qsh[64:128, :], in_=qsb[0:64, :])
                        t1 = f16t()
                        nc.vector.tensor_tensor(out=t1[:], in0=qsb[:],
                                                in1=cosS[:, c0:c1],
                                                op=Alu.mult)
                        t2 = f16t()
                        nc.vector.tensor_tensor(out=t2[:], in0=qsh[:],
                                                in1=sinS[:, c0:c1],
                                                op=Alu.mult)
                        nc.vector.tensor_tensor(out=dst[:, c0:c1], in0=t1[:],
                                                in1=t2[:], op=Alu.add)
                    else:
                        vt = f16t()
                        nc.scalar.activation(vt[:], mm[:], Act.Copy)
                        # defer transposes one chunk so PE isn't blocked
                        # waiting on the ACT copy
                        vt_pend.append((vt, ch))
                        if len(vt_pend) > 1:
                            _v_transpose(nc, Act, ptr, id16, Vn,
                                         *vt_pend.pop(0))
            # --- attention for this head: oT[d, s] directly ---
            for qs in range(NCHUNK):
                if vt_pend:
                    _v_transpose(nc, Act, ptr, id16, Vn, *vt_pend.pop(0))
                c0, c1 = qs * 512, (qs + 1) * 512
                npairs = 4 * qs + 4
                pv_ps = pv.tile([128, 512], F32, tag="pv", name="pv")
                z_ps = pz.tile([1, 512], F32, tag="zrow", name="z")
                sc_pend = []
                for jj in range(npairs):
                    scp = pp.tile([128, 512], F32, tag="mm", name="mmsc")
                    nc.tensor.matmul(scp[:], kT[:, jj * 128:(jj + 1) * 128],
                                     qT[:, c0:c1], start=True, stop=True)
                    sc_pend.append((scp, jj))
                    if len(sc_pend) > 1:
                        _pv_consume(nc, Act, Alu, w6, BF16, maskt, Vn, onesbf,
                                    pv_ps, z_ps, qs, npairs,
                                    *sc_pend.pop(0))
                _pv_consume(nc, Act, Alu, w6, BF16, maskt, Vn, onesbf,
                            pv_ps, z_ps, qs, npairs, *sc_pend.pop(0))
                # normalize: oT[:, h*S+qs] = pv * (1/z) broadcast
                rzr = w3.tile([1, 512], F32, tag="rows", name="rz")
                nc.vector.reciprocal(rzr[:], z_ps[:])
                nc.sync.dma_start(out=rz_d[0:1, h * S + c0:h * S + c1],
                                  in_=rzr[:])
                rzB = w2.tile([128, 512], F32, tag="bc32", name="rzB")
                nc.scalar.dma_start(
                    out=rzB[:],
                    in_=rz_d[0:1, h * S + c0:h * S + c1].unsqueeze(1)
                    .broadcast_to([1, 128, 512]))
                oTt = f16t()
                nc.vector.tensor_tensor(out=oTt[:], in0=pv_ps[:],
                                        in1=rzB[:], op=Alu.mult)
                nc.gpsimd.dma_start(out=oT_d[h * 128:(h + 1) * 128, c0:c1],
                                    in_=oTt[:])
        if debug:
            nc.sync.dma_start(out=dbg["qT_dbg"][:], in_=qT[:])
            nc.sync.dma_start(out=dbg["kT_dbg"][:], in_=kT[:])
            nc.sync.dma_start(out=dbg["oT_dbg"][:], in_=oT_d[:])
            nc.sync.dma_start(out=dbg["rowsc_dbg"][0:1, 0:S], in_=rowsc_d[0:1, 0:S])
            nc.gpsimd.dma_start(out=dbg["rowsc_dbg"][0:1, S:(1 + NHC) * S], in_=rz_d[:])
        close_pool("xph")

        # ============ P2: o-proj, ch-outer + chunked RS1 ==========
        op2 = open_pool(name="oTp", bufs=1)
        wo_all = op2.tile([128, NKT * 512], F16, tag="wo_all")
        for ot in range(NKT):
            dequant_slab(qw_o_d[:], sc_o_d[:], ot, NHC, F16,
                         wo_all[:, ot * 512:(ot + 1) * 512])
        for ch in range(NCHUNK):
            c0, c1 = ch * 512, (ch + 1) * 512
            oTc = []
            rs_ps = prs.tile([32, 512], F32, tag="rs", name="rsop")
            for kt in range(NHC):
                ott = f16t()
                nc.scalar.dma_start(out=ott[:],
                                    in_=oT_d[kt * 128:(kt + 1) * 128, c0:c1])
                oTc.append(ott)
                nc.tensor.matmul(rs_ps[:], esw16[:, 31 - kt:63 - kt],
                                 ott[:], start=(kt == 0), stop=(kt == NHC - 1))
            rso_c = f16t()
            nc.scalar.activation(rso_c[0:32, :], rs_ps[:], Act.Copy)
            for ot in range(NKT):
                zl = load_zl(nz_o_d[:], ot, NHC, F16)
                mm = pp.tile([128, 512], F32, tag="mm", name="mmo")
                for kt in range(NHC):
                    nc.tensor.matmul(
                        mm[:], wo_all[:, ot * 512 + kt * 128:
                                      ot * 512 + (kt + 1) * 128],
                        oTc[kt][:], start=(kt == 0), stop=False)
                nc.tensor.matmul(mm[:], zl[0:NHC, :], rso_c[0:NHC, :],
                                 start=False, stop=True)
                pt = f16t(BF16)
                nc.scalar.activation(pt[:], mm[:], Act.Copy)
                nc.gpsimd.dma_start(
                    out=part1_d[ch][ot * 128:(ot + 1) * 128, :], in_=pt[:])
            nc.gpsimd.collective_compute(
                "ReduceScatter", Alu.add,
                replica_groups=[list(range(NCORES))],
                ins=[part1_d[ch][:].opt()], outs=[rs1o_d[ch][:].opt()])
        close_pool("oTp")

        # ============ P2.5: residual + rms2 on own slice (AG order) ========
        hbp = open_pool(name="h2bp", bufs=1)
        h2b = hbp.tile([128, NHC * S], BF16, tag="h2b")
        hp = open_pool(name="h2ph", bufs=1)
        h2s = hp.tile([128, NHC * S], F32, tag="h2s")

        for ch in range(NCHUNK):
            c0, c1 = ch * 512, (ch + 1) * 512
            ssq_ps = pz.tile([1, 512], F32, tag="zrow", name="ssq2")
            for rg in range(NHC):
                ta = f16t(BF16)
                nc.gpsimd.dma_start(out=ta[:],
                                    in_=rs1o_d[ch][rg * 128:(rg + 1) * 128, :])
                th = f32t()
                nc.sync.dma_start(out=th[:],
                                  in_=hsl_d[rg * 128:(rg + 1) * 128, c0:c1])
                h2sl = h2s[:, rg * S + c0: rg * S + c1]
                nc.vector.tensor_tensor(out=h2sl, in0=th[:], in1=ta[:],
                                        op=Alu.add)
                nc.vector.tensor_scalar(
                    out=h2b[:, rg * S + c0: rg * S + c1], in0=h2sl,
                    scalar1=1.0, scalar2=None, op0=Alu.mult)
                sq = f16t()
                nc.scalar.activation(sq[:], h2sl, Act.Square)
                nc.tensor.matmul(ssq_ps[:], ones16[:], sq[:],
                                 start=(rg == 0), stop=(rg == NHC - 1))
            srow = w3.tile([1, 512], F32, tag="rows", name="sr2")
            nc.scalar.activation(srow[:], ssq_ps[:], Act.Copy)
            nc.sync.dma_start(out=ssq2p_d[0:1, c0:c1], in_=srow[:])
        nc.gpsimd.collective_compute(
            "AllReduce", Alu.add, replica_groups=[list(range(NCORES))],
            ins=[ssq2p_d[:].opt()], outs=[ssq2s_d[:].opt()])
        rstd_from(ssq2s_d, 5 * S)
        for ch in range(NCHUNK):
            c0, c1 = ch * 512, (ch + 1) * 512
            rstdB = bcast(5 * S, c0, c1, "rstdB2")
            for rg in range(NHC):
                h2sl = h2s[:, rg * S + c0: rg * S + c1]
                x2t = f16t(BF16)
                nc.vector.scalar_tensor_tensor(
                    out=x2t[:], in0=h2sl, scalar=ln2s[:, rg:rg + 1],
                    in1=rstdB[:], op0=Alu.mult, op1=Alu.mult)
                nc.sync.dma_start(
                    out=x2own_d[rg * 128:(rg + 1) * 128, c0:c1], in_=x2t[:])
        if debug:
            for rg in range(NHC):
                for ch in range(NCHUNK):
                    c0, c1 = ch * 512, (ch + 1) * 512
                    dbt = f32t()
                    nc.vector.tensor_scalar(
                        out=dbt[:], in0=h2s[:, rg * S + c0: rg * S + c1],
                        scalar1=1.0, scalar2=None, op0=Alu.mult)
                    nc.sync.dma_start(
                        out=dbg["h2s_dbg"][rg * 128:(rg + 1) * 128, c0:c1],
                        in_=dbt[:])
            nc.sync.dma_start(out=dbg["x2own_dbg"][:], in_=x2own_d[:])
        close_pool("h2ph")
        nc.gpsimd.collective_compute(
            "AllGather", Alu.bypass, replica_groups=[list(range(NCORES))],
            ins=[x2own_d[:].opt()], outs=[x2all_d[:].opt()])

        # =================== P3: MLP gate/up (slab-outer) ===================
        ryp = open_pool(name="ryp", bufs=1)
        rsy = ryp.tile([NIT, S], BF16, tag="rsy")
        rsx2 = ryp.tile([32, S], BF16, tag="rsx2")
        mp = open_pool(name="mlp", bufs=1)
        x2T = mp.tile([128, NKT * S], BF16, tag="x2T")

        for ch in range(NCHUNK):
            c0, c1 = ch * 512, (ch + 1) * 512
            rsx_ps = prs.tile([32, 512], F32, tag="rs", name="rsx2p")
            for kt in range(NKT):
                xsl = x2T[:, kt * S + c0: kt * S + c1]
                nc.scalar.dma_start(out=xsl,
                                    in_=x2all_d[kt * 128:(kt + 1) * 128,
                                                c0:c1])
                nc.tensor.matmul(rsx_ps[:], eswbf[:, 31 - kt:63 - kt],
                                 xsl, start=(kt == 0), stop=(kt == NKT - 1))
            nc.scalar.activation(rsx2[:, c0:c1], rsx_ps[:], Act.Copy)

        for it in range(NIT):
            wg = wsl.tile([128, NKT * 128], BF16, tag="w16", name="wg")
            dequant_slab(qw_gu_d[:], sc_gu_d[:], it, NKT, BF16, wg)
            zlg = load_zl(nz_gu_d[:], it, 32, BF16)
            wu = wsl.tile([128, NKT * 128], BF16, tag="w16", name="wu")
            dequant_slab(qw_gu_d[:], sc_gu_d[:], NIT + it, NKT, BF16, wu)
            zlu = load_zl(nz_gu_d[:], NIT + it, 32, BF16)
            for ch in range(NCHUNK):
                c0, c1 = ch * 512, (ch + 1) * 512
                gp = pp.tile([128, 512], F32, tag="mm", name="mmg")
                for kt in range(NKT):
                    nc.tensor.matmul(
                        gp[:], wg[:, kt * 128:(kt + 1) * 128],
                        x2T[:, kt * S + c0: kt * S + c1],
                        start=(kt == 0), stop=False)
                nc.tensor.matmul(gp[:], zlg[0:32, :], rsx2[:, c0:c1],
                                 start=False, stop=True)
                up = pv.tile([128, 512], F32, tag="pv", name="mmu")
                for kt in range(NKT):
                    nc.tensor.matmul(
                        up[:], wu[:, kt * 128:(kt + 1) * 128],
                        x2T[:, kt * S + c0: kt * S + c1],
                        start=(kt == 0), stop=False)
                nc.tensor.matmul(up[:], zlu[0:32, :], rsx2[:, c0:c1],
                                 start=False, stop=True)
                sg = f32t()
                nc.scalar.activation(sg[:], gp[:], Act.Silu)
                yt = f16t(BF16)
                nc.vector.tensor_tensor(out=yt[:], in0=sg[:], in1=up[:],
                                        op=Alu.mult)
                rs_ps = pz.tile([1, 512], F32, tag="zrow", name="rsyp")
                nc.tensor.matmul(rs_ps[:], onesbf[:], yt[:],
                                 start=True, stop=True)
                rsr = w3.tile([1, 512], BF16, tag="rowsh", name="rsyr")
                nc.scalar.activation(rsr[:], rs_ps[:], Act.Copy)
                nc.sync.dma_start(out=rsy[it:it + 1, c0:c1], in_=rsr[:])
                nc.gpsimd.dma_start(
                    out=yT_d[it * 128:(it + 1) * 128, c0:c1], in_=yt[:])
        close_pool("mlp")

        # ---- down, ot-outer + chunked RS2 over row groups ----
        dnp = open_pool(name="dnp", bufs=1)
        yT = dnp.tile([128, NIT * S], BF16, tag="yT")
        for kt in range(NIT):
            for ch in range(NCHUNK):
                c0, c1 = ch * 512, (ch + 1) * 512
                nc.scalar.dma_start(
                    out=yT[:, kt * S + c0: kt * S + c1],
                    in_=yT_d[kt * 128:(kt + 1) * 128, c0:c1])
        for ot in range(NKT):
            w16d = wsl.tile([128, NKT * 128], BF16, tag="w16", name="wdn")
            dequant_slab(qw_dn_d[:], sc_dn_d[:], ot, NIT, BF16, w16d)
            zl = load_zl(nz_dn_d[:], ot, NIT, BF16)
            r, ri = ot // 8, ot % 8
            for ch in range(NCHUNK):
                c0, c1 = ch * 512, (ch + 1) * 512
                mm = pp.tile([128, 512], F32, tag="mm", name="mmd")
                for kt in range(NIT):
                    nc.tensor.matmul(
                        mm[:], w16d[:, kt * 128:(kt + 1) * 128],
                        yT[:, kt * S + c0: kt * S + c1],
                        start=(kt == 0), stop=False)
                nc.tensor.matmul(mm[:], zl[0:NIT, :], rsy[:, c0:c1],
                                 start=False, stop=True)
                # own-row residual: h2b rows (rg = ot%4) masked per core
                pt = f16t(BF16)
                nc.vector.scalar_tensor_tensor(
                    out=pt[:], in0=h2b[:, (ot % 4) * S + c0: (ot % 4) * S + c1],
                    scalar=hmask[:, ot:ot + 1], in1=mm[:],
                    op0=Alu.mult, op1=Alu.add)
                nc.sync.dma_start(
                    out=part2_d[r][ri * 128:(ri + 1) * 128, c0:c1], in_=pt[:])
            if ri == 7:
                nc.gpsimd.collective_compute(
                    "ReduceScatter", Alu.add,
                    replica_groups=[list(range(NCORES))],
                    ins=[part2_d[r][:].opt()], outs=[rs2o_d[r][:].opt()])
        if debug:
            nc.sync.dma_start(out=dbg["rsy_dbg"][:], in_=rsy[:])
        close_pool("dnp")
        close_pool("ryp")
        close_pool("h2bp")

        # ---- final: convert RS2 outputs to f32 and store ----
        for r in range(4):
            for ch in range(NCHUNK):
                c0, c1 = ch * 512, (ch + 1) * 512
                tb = f16t(BF16)
                nc.gpsimd.dma_start(out=tb[:], in_=rs2o_d[r][:, c0:c1])
                tf = f32t()
                nc.vector.tensor_scalar(out=tf[:], in0=tb[:], scalar1=1.0,
                                        scalar2=None, op0=Alu.mult)
                nc.sync.dma_start(out=outT_d[r * 128:(r + 1) * 128, c0:c1],
                                  in_=tf[:])

        for cm, nm in reversed(ctx_pools):
            cm.__exit__(None, None, None)
        ctx_pools.clear()

    nc.compile()
    return nc


def _v_transpose(nc, Act, ptr, id16, Vn, vt, ch):
    import concourse.mybir as mybir
    F16 = mybir.dt.float16
    for st4 in range(4):
        st = ch * 4 + st4
        trp = ptr.tile([128, 128], F16, tag="tr", name="trv")
        nc.tensor.transpose(trp[:], vt[:, st4 * 128:(st4 + 1) * 128], id16[:])
        nc.scalar.activation(Vn[:, st * 128:(st + 1) * 128], trp[:], Act.Copy)


def _pv_consume(nc, Act, Alu, w6, BF16, maskt, Vn, onesbf, pv_ps, z_ps,
                qs, npairs, scp, jj):
    ET = w6.tile([128, 512], BF16, tag="f16t", name="ET")
    nc.scalar.activation(ET[:], scp[:], Act.Exp, scale=SCALE)
    if jj >= 4 * qs:
        dd = jj - 4 * qs
        nc.vector.tensor_tensor(out=ET[:], in0=ET[:],
                                in1=maskt[:, 384 - 128 * dd: 896 - 128 * dd],
                                op=Alu.mult)
    nc.tensor.matmul(pv_ps[:], Vn[:, jj * 128:(jj + 1) * 128], ET[:],
                     start=(jj == 0), stop=(jj == npairs - 1))
    nc.tensor.matmul(z_ps[:], onesbf[:], ET[:],
                     start=(jj == 0), stop=(jj == npairs - 1))


def _host_prep(inputs):
    """Build the 8 per-core input maps from full inputs."""
    import ml_dtypes
    bf16 = ml_dtypes.bfloat16
    f16 = np.float16

    def unpack_z1(qz):
        sh = (np.arange(8, dtype=np.uint32) * 4)
        z = ((qz[:, :, None].view(np.uint32) >> sh[None, None, :]) & 15)
        return z.reshape(qz.shape[0], -1).astype(np.float32) + 1.0

    h = np.asarray(inputs["hidden_states"], np.float32)[0]     # [S, HID]
    hT = np.ascontiguousarray(h.T)                             # [HID, S]
    hb = hT.astype(bf16)
    sin = np.asarray(inputs["sin"], np.float32)                # [S, HD]
    cos = np.asarray(inputs["cos"], np.float32)
    cosT = np.ascontiguousarray(cos.T).astype(f16)
    sinf = sin.T.copy()
    sinf[0:64, :] *= -1.0                                      # rot-half fold
    sinfT = np.ascontiguousarray(sinf).astype(f16)

    qkv_qw, qkv_sc, qkv_nz = [], [], []
    for nm in ("q", "k", "v"):
        qw = np.asarray(inputs["qw_" + nm])
        sc = np.asarray(inputs["sc_" + nm], np.float32)
        z1 = unpack_z1(np.asarray(inputs["qz_" + nm]))
        qkv_qw.append(qw); qkv_sc.append(sc); qkv_nz.append(-(z1 * sc))

    qw_o = np.asarray(inputs["qw_o"])
    sc_o = np.asarray(inputs["sc_o"], np.float32)
    nz_o = -(unpack_z1(np.asarray(inputs["qz_o"])) * sc_o)

    def pad_cols(a, w):
        out = np.zeros((a.shape[0], w), a.dtype)
        out[:, :a.shape[1]] = a
        return out

    qw_g = pad_cols(np.asarray(inputs["qw_gate"]), IPAD)
    qw_u = pad_cols(np.asarray(inputs["qw_up"]), IPAD)
    sc_g = pad_cols(np.asarray(inputs["sc_gate"], np.float32), IPAD)
    sc_u = pad_cols(np.asarray(inputs["sc_up"], np.float32), IPAD)
    nz_g = pad_cols(-(unpack_z1(np.asarray(inputs["qz_gate"]))
                      * np.asarray(inputs["sc_gate"], np.float32)), IPAD)
    nz_u = pad_cols(-(unpack_z1(np.asarray(inputs["qz_up"]))
                      * np.asarray(inputs["sc_up"], np.float32)), IPAD)

    qw_dn = np.zeros((IPAD // 8, HID), np.int32)
    qw_dn[:INTER // 8] = np.asarray(inputs["qw_down"])
    sc_dn = np.zeros((IPAD // GS, HID), np.float32)
    sc_dn[:INTER // GS] = np.asarray(inputs["sc_down"], np.float32)
    nz_dn = np.zeros((IPAD // GS, HID), np.float32)
    nz_dn[:INTER // GS] = -(unpack_z1(np.asarray(inputs["qz_down"]))
                            * np.asarray(inputs["sc_down"], np.float32))

    ln1 = np.asarray(inputs["ln1_w"], np.float32)
    ln2 = np.asarray(inputs["ln2_w"], np.float32)

    def repack_qw(qw):
        nkt = qw.shape[0] // 16
        notile = qw.shape[1] // 128
        return np.ascontiguousarray(
            qw.reshape(nkt, 16, notile, 128).transpose(2, 1, 0, 3)
            .reshape(notile * 16, nkt * 128))

    def repack_sc(sc):
        G = sc.shape[0]
        notile = sc.shape[1] // 128
        return np.ascontiguousarray(
            sc.reshape(G, notile, 128).transpose(1, 0, 2)
            .reshape(notile * G, 128))

    maps = []
    for c in range(NCORES):
        cs = slice(c * OPC, (c + 1) * OPC)
        isl = slice(c * IPC, (c + 1) * IPC)
        hsl = hT[c * OPC:(c + 1) * OPC]
        ln2p = ln2[c * OPC:(c + 1) * OPC]
        hm = np.zeros((128, NKT), np.float32)
        for ot in range(NKT):
            if ot // 4 == c:
                hm[:, ot] = 1.0
        m = {
            "hb": hb, "hsl": np.ascontiguousarray(hsl),
            "cosT": cosT, "sinfT": sinfT, "ln1": ln1,
            "ln2p": np.ascontiguousarray(ln2p), "hmask": hm,
            "qw_qkv": repack_qw(
                np.concatenate([qkv_qw[i][:, cs] for i in range(3)], axis=1)),
            "sc_qkv": repack_sc(
                np.concatenate([qkv_sc[i][:, cs] for i in range(3)],
                               axis=1).astype(f16)),
            "nz_qkv": np.ascontiguousarray(
                np.concatenate([qkv_nz[i][:, cs] for i in range(3)],
                               axis=1)).astype(f16),
            "qw_o": repack_qw(qw_o[c * OPC // 8:(c + 1) * OPC // 8]),
            "sc_o": repack_sc(
                sc_o[c * OPC // GS:(c + 1) * OPC // GS].astype(f16)),
            "nz_o": np.ascontiguousarray(
                nz_o[c * OPC // GS:(c + 1) * OPC // GS]).astype(f16),
            "qw_gu": repack_qw(
                np.concatenate([qw_g[:, isl], qw_u[:, isl]], axis=1)),
            "sc_gu": repack_sc(
                np.concatenate([sc_g[:, isl], sc_u[:, isl]],
                               axis=1).astype(bf16)),
            "nz_gu": np.ascontiguousarray(
                np.concatenate([nz_g[:, isl], nz_u[:, isl]],
                               axis=1)).astype(bf16),
            "qw_dn": repack_qw(qw_dn[c * IPC // 8:(c + 1) * IPC // 8]),
            "sc_dn": repack_sc(
                sc_dn[c * NIT:(c + 1) * NIT].astype(bf16)),
            "nz_dn": np.ascontiguousarray(
                nz_dn[c * NIT:(c + 1) * NIT]).astype(bf16),
        }
        maps.append(m)
    return maps


def run(inputs, debug=False, trace=False):
    from concourse.bass_utils import run_bass_kernel_spmd
    key = "dbg" if debug else "rel"
    if key not in _BUILD_CACHE:
        _BUILD_CACHE[key] = _build(debug=debug)
    nc = _BUILD_CACHE[key]
    maps = _host_prep(inputs)
    res = run_bass_kernel_spmd(nc, maps, core_ids=list(range(NCORES)),
                               trace=trace)
    # outT rows of core c, piece r = original feature rows (8r + c)*128
    full = np.zeros((HID, S), np.float32)
    for c in range(NCORES):
        oc = res.results[c]["outT"]
        for r in range(4):
            full[(8 * r + c) * 128:(8 * r + c + 1) * 128] = \
                oc[r * 128:(r + 1) * 128]
    out = np.ascontiguousarray(full.T)[None]
    return out, res


def kernel(**inputs):
    out, _ = run(inputs)
    return out
